# revision 2
# baseline (speedup 1.0000x reference)
"""Trainium2 Bass kernel for nn_DenoiserPairFeatures.

Math: the [n,n,219] feature tensor is a concat of one-hots (seq-sep 127,
dist-bins 30+30) plus zero blocks, so feats @ W.T + b collapses to 3 table
gathers + bias.  Gathers are realized on the TensorEngine as sign-step
matmuls with host-precomputed compensated cumulative bf16 tables (hi+lo
split; error does not accumulate along a chain).

Seq-sep band trick: for a given row i the sep one-hot varies only inside a
256-wide j-window around i (the "band"); outside it the sep contribution
is a constant +/-Qsep.  Each row's j-tiles are processed in a rotated
order so the band is always tiles 0,1: those get the full 3-matmul stack
(sep-hi, sep-lo, bins), the other six need only the single 124-row "B"
matmul whose extra sign-rows (thresholded on 128*jb - j) add +/-Qsep/2
pairs and the 4-way-split bias B0.  The host un-rotates the output rows.

LayerNorm is fused: bn_stats/bn_aggr per 128-pair tile, applied as
out = y*scale + (-mean*scale) in one activation/tensor_scalar pass with
the pair mask folded into the scale.  Rows with mask[i]==0 are written
as zeros by plain DMA without compute; active rows are distributed
round-robin over the 8 cores so the SPMD program only runs R =
ceil(n_active/8) compute slots.
"""

import os
import sys

sys.path.insert(0, "/opt/trn_rl_repo")

import numpy as np
import ml_dtypes

N = 1024
SEQ = 127          # seq-sep one-hot classes
NB = 30            # dist bins
C_OUT = 256
N_CORES = 8
JT = 8             # j-tiles per row (1024 / 128)
LN_EPS = 1e-5

BF16 = ml_dtypes.bfloat16

_PROGRAM_CACHE = {}
LAST_PROFILE = None  # set when KERNEL_TRACE=1


def _bf16_f64(x):
    return np.asarray(x, np.float64).astype(BF16).astype(np.float64)


def _comp_chain(T):
    """Compensated half-delta chain for sign-step gather, split hi+lo bf16.

    T: [M+1, C] float64 exact targets.  Returns (Ghi, Glo [M, C] float64 of
    bf16-representable values).  Realized partial sums
    P(k) = 2*sum_{m<=k} (Ghi+Glo)[m] track T[k]-T[0] with non-accumulating
    ~bf16^2-level error.
    """
    M = T.shape[0] - 1
    C = T.shape[1]
    P = np.zeros(C, np.float64)
    Ghi = np.empty((M, C), np.float64)
    Glo = np.empty((M, C), np.float64)
    for k in range(1, M + 1):
        g = (T[k] - T[0] - P) * 0.5
        ghi = _bf16_f64(g)
        glo = _bf16_f64(g - ghi)
        Ghi[k - 1] = ghi
        Glo[k - 1] = glo
        P += 2.0 * (ghi + glo)
    return Ghi, Glo


def _split4(v):
    p1 = _bf16_f64(v)
    p2 = _bf16_f64(v - p1)
    p3 = _bf16_f64(v - p1 - p2)
    p4 = _bf16_f64(v - p1 - p2 - p3)
    return p1, p2, p3, p4


def _split2(v):
    p1 = _bf16_f64(v)
    p2 = _bf16_f64(v - p1)
    return p1, p2


def _dist_bins(coords):
    """Bin indices exactly as the reference computes them (same jnp ops on
    the default backend, so borderline fp32 decisions match bit-for-bit)."""
    import jax.numpy as jnp

    edges = jnp.linspace(0.1, 3.0, NB - 1)
    x = jnp.asarray(np.asarray(coords, np.float32))
    diff = x[:, None, :] - x[None, :, :]
    d = jnp.sqrt(jnp.sum(jnp.square(diff), axis=-1) + 1e-10)
    return np.asarray(jnp.searchsorted(edges, d), dtype=np.int32)


def _build_tables(W, b):
    """Returns ga_hi, ga_lo [128, 256] (sep chains) and gb [124, 256]:
    bins hi, bins lo, +Qsep/2 (hi,lo), -Qsep/2 (hi,lo), B0 4-way split."""
    W = np.asarray(W, np.float64)
    b = np.asarray(b, np.float64)
    Tsep = W[:, 0:SEQ].T.copy()            # [127, 256]
    Tt = W[:, SEQ:SEQ + NB].T.copy()       # [30, 256]
    Tsc = W[:, SEQ + NB:SEQ + 2 * NB].T.copy()
    Gsep_h, Gsep_l = _comp_chain(Tsep)     # [126, 256]
    Gt_h, Gt_l = _comp_chain(Tt)           # [29, 256]
    Gsc_h, Gsc_l = _comp_chain(Tsc)        # [29, 256]
    Qsep = (Gsep_h + Gsep_l).sum(axis=0)
    Qt = (Gt_h + Gt_l).sum(axis=0)
    Qsc = (Gsc_h + Gsc_l).sum(axis=0)
    B0 = b + Tsep[0] + Tt[0] + Tsc[0] + Qsep + Qt + Qsc

    zero = np.zeros((1, C_OUT))
    ga_hi = np.concatenate([Gsep_h, zero, zero], axis=0)   # [128, 256]
    ga_lo = np.concatenate([Gsep_l, zero, zero], axis=0)   # [128, 256]

    qp1, qp2 = _split2(0.5 * Qsep)
    qm1, qm2 = _split2(-0.5 * Qsep)
    b1, b2, b3, b4 = _split4(B0)
    gb = np.concatenate(
        [Gt_h, Gsc_h, Gt_l, Gsc_l,                         # 0..115
         qp1[None], qp2[None], qm1[None], qm2[None],       # 116..119
         b1[None], b2[None], b3[None], b4[None]], axis=0)  # 120..123
    return ga_hi.astype(BF16), ga_lo.astype(BF16), gb.astype(BF16)


def _build_program(R, n_zero_rows):
    """Build + compile the SPMD program for R active row-slots."""
    key = (R, n_zero_rows)
    if key in _PROGRAM_CACHE:
        return _PROGRAM_CACHE[key]

    from concourse import bacc, mybir, tile

    dt = mybir.dt
    nc = bacc.Bacc("TRN2", target_bir_lowering=False, debug=False,
                   num_devices=N_CORES)

    gah_d = nc.dram_tensor("ga_hi", [128, C_OUT], dt.bfloat16, kind="ExternalInput").ap()
    gal_d = nc.dram_tensor("ga_lo", [128, C_OUT], dt.bfloat16, kind="ExternalInput").ap()
    gb_d = nc.dram_tensor("gb", [124, C_OUT], dt.bfloat16, kind="ExternalInput").ap()
    lta_d = nc.dram_tensor("lta", [4, 128 * 128], dt.bfloat16, kind="ExternalInput").ap()
    ltb_d = nc.dram_tensor("ltb", [6, 128 * 128], dt.bfloat16, kind="ExternalInput").ap()
    rowdat_d = nc.dram_tensor("rowdat", [6, 128 * 1280], dt.bfloat16, kind="ExternalInput").ap()
    biasa_d = nc.dram_tensor("biasa", [128, 1], dt.float32, kind="ExternalInput").ap()
    biasb_d = nc.dram_tensor("biasb", [124, 1], dt.float32, kind="ExternalInput").ap()
    pmt_d = nc.dram_tensor("pmt", [128, 1024], dt.float32, kind="ExternalInput").ap()
    out_d = nc.dram_tensor("out", [128, 1024, C_OUT], dt.float32, kind="ExternalOutput").ap()

    with tile.TileContext(nc) as tc:
        with (
            tc.tile_pool(name="const", bufs=1) as cpool,
            tc.tile_pool(name="fa", bufs=6) as fapool,
            tc.tile_pool(name="fb", bufs=6) as fbpool,
            tc.tile_pool(name="pbc", bufs=4, space="PSUM") as pbc,
            tc.tile_pool(name="py", bufs=4, space="PSUM") as pyp,
            tc.tile_pool(name="stat", bufs=8) as spool,
            tc.tile_pool(name="fin", bufs=6) as finpool,
            tc.tile_pool(name="ot", bufs=4) as opool,
        ):
            GAH = cpool.tile([128, C_OUT], dt.bfloat16)
            nc.sync.dma_start(out=GAH[:], in_=gah_d[:])
            GAL = cpool.tile([128, C_OUT], dt.bfloat16)
            nc.sync.dma_start(out=GAL[:], in_=gal_d[:])
            GB = cpool.tile([124, C_OUT], dt.bfloat16)
            nc.sync.dma_start(out=GB[:], in_=gb_d[:])
            LTA = cpool.tile([4, 128 * 128], dt.bfloat16)
            nc.sync.dma_start(out=LTA[:], in_=lta_d[:])
            LTB = cpool.tile([6, 128 * 128], dt.bfloat16)
            nc.sync.dma_start(out=LTB[:], in_=ltb_d[:])
            BIASA = cpool.tile([128, 1], dt.float32)
            nc.sync.dma_start(out=BIASA[:], in_=biasa_d[:])
            BIASB = cpool.tile([124, 1], dt.float32)
            nc.sync.dma_start(out=BIASB[:], in_=biasb_d[:])
            PMT = cpool.tile([128, 1024], dt.float32)
            nc.sync.dma_start(out=PMT[:], in_=pmt_d[:])
            ZT = cpool.tile([128, JT * C_OUT], dt.float32)
            nc.vector.memset(ZT[:], 0.0)
            EPS = cpool.tile([128, 1], dt.float32)
            nc.vector.memset(EPS[:], LN_EPS)

            Sign = mybir.ActivationFunctionType.Sign
            Sqrt = mybir.ActivationFunctionType.Sqrt
            Ident = mybir.ActivationFunctionType.Identity
            mult = mybir.AluOpType.mult
            add = mybir.AluOpType.add

            for r in range(R):
                # ---- stage per-row data from DRAM ----
                RD = fapool.tile([6, 1280], dt.bfloat16, tag="rd")
                nc.sync.dma_start(out=RD[:], in_=rowdat_d[:, r * 1280:(r + 1) * 1280])
                TBS = RD[:, 0:1024]
                ARH = RD[0:4, 1024:1280]

                # ---- broadcast matmuls + sign steps -> F matrices ----
                FA = fapool.tile([128, 256], dt.bfloat16, tag="fa")
                FB = fbpool.tile([124, 1024], dt.bfloat16, tag="fb")
                PA = pbc.tile([128, 256], dt.float32, tag="pbc")
                nc.tensor.matmul(PA[:], LTA[:, r * 128:(r + 1) * 128],
                                 ARH, start=True, stop=True)
                nc.scalar.activation(FA[:], PA[:], Sign, bias=BIASA[:, 0:1])
                for h in range(2):
                    PB = pbc.tile([128, 512], dt.float32, tag="pbc")
                    nc.tensor.matmul(
                        PB[0:124, :], LTB[:, r * 128: r * 128 + 124],
                        TBS[:, h * 512:(h + 1) * 512], start=True, stop=True)
                    nc.scalar.activation(
                        FB[:, h * 512:(h + 1) * 512], PB[0:124, :], Sign,
                        bias=BIASB[:, 0:1])

                # ---- main matmuls (bank-paired Y) + stats + apply ----
                MV = spool.tile([128, JT, 2], dt.float32, tag="mv")
                SD = finpool.tile([128, JT], dt.float32, tag="sd")
                BD = finpool.tile([128, JT], dt.float32, tag="bd")
                OT = opool.tile([128, JT * C_OUT], dt.float32, tag="ot")
                ypairs = []
                for jp in range(JT // 2):
                    Y2 = pyp.tile([128, 2, C_OUT], dt.float32, tag="y")
                    ypairs.append(Y2)
                    for s in range(2):
                        jc = 2 * jp + s
                        if jc < 2:
                            nc.tensor.matmul(
                                Y2[:, s, :], FA[:, jc * 128:(jc + 1) * 128],
                                GAH[:], start=True, stop=False)
                            nc.tensor.matmul(
                                Y2[:, s, :], FA[:, jc * 128:(jc + 1) * 128],
                                GAL[:], start=False, stop=False)
                            nc.tensor.matmul(
                                Y2[:, s, :], FB[:, jc * 128:(jc + 1) * 128],
                                GB[:], start=False, stop=True)
                        else:
                            nc.tensor.matmul(
                                Y2[:, s, :], FB[:, jc * 128:(jc + 1) * 128],
                                GB[:], start=True, stop=True)
                    ST = spool.tile([128, 2, 6], dt.float32, tag="st")
                    nc.vector.bn_stats(ST[:, 0, :], Y2[:, 0, :])
                    nc.vector.bn_stats(ST[:, 1, :], Y2[:, 1, :])
                    nc.vector.bn_aggr(MV[:, 2 * jp, :], ST[:, 0, :])
                    nc.vector.bn_aggr(MV[:, 2 * jp + 1, :], ST[:, 1, :])

                    if jp % 2 == 1:
                        g0 = 2 * (jp - 1)   # first jc of the 4-tile group
                        g1 = g0 + 4
                        # scale = pm / sqrt(var+eps); bias2 = -mean*scale
                        T0 = finpool.tile([128, 4], dt.float32, tag="t0")
                        nc.scalar.activation(
                            T0[:], MV[:, g0:g1, 1], Sqrt, bias=EPS[:, 0:1])
                        T1 = finpool.tile([128, 4], dt.float32, tag="t1")
                        nc.vector.reciprocal(T1[:], T0[:])
                        nc.vector.tensor_tensor(
                            SD[:, g0:g1], T1[:],
                            PMT[:, r * JT + g0: r * JT + g1], op=mult)
                        nc.vector.scalar_tensor_tensor(
                            BD[:, g0:g1], MV[:, g0:g1, 0], -1.0, SD[:, g0:g1],
                            op0=mult, op1=mult)
                        for j2 in range(g0, g1):
                            ysrc = ypairs[j2 // 2][:, j2 % 2, :]
                            odst = OT[:, j2 * C_OUT:(j2 + 1) * C_OUT]
                            if j2 % 4 == 0:
                                nc.vector.tensor_scalar(
                                    odst, ysrc,
                                    SD[:, j2:j2 + 1], BD[:, j2:j2 + 1],
                                    op0=mult, op1=add)
                            else:
                                nc.scalar.activation(
                                    odst, ysrc, Ident,
                                    bias=BD[:, j2:j2 + 1], scale=SD[:, j2:j2 + 1])
                        half = (jp - 1) // 2
                        nc.sync.dma_start(
                            out=out_d[r, half * 512:(half + 1) * 512, :]
                                .rearrange("(jc p) o -> p jc o", p=128),
                            in_=OT[:, half * 4 * C_OUT:(half + 1) * 4 * C_OUT]
                                .rearrange("p (jc o) -> p jc o", o=C_OUT))

            # ---- zero rows: broadcast DMAs chunked across queues ----
            zr = R
            while zr < 128:
                ze = min(zr + 4, 128)
                nzc = ze - zr
                nc.sync.dma_start(
                    out=out_d[zr:ze].rearrange("z (jc p) o -> p (z jc) o", p=128),
                    in_=ZT[:, 0:C_OUT].rearrange("p (u o) -> p u o", u=1)
                        .to_broadcast([128, nzc * JT, C_OUT]))
                zr = ze

    nc.compile()
    _PROGRAM_CACHE[key] = nc
    return nc


def _host_data(mask, x_t, x_sc, W, b):
    """Everything data-dependent: bins, tables, row assignment (actives
    first, round-robin over cores), per-row j-rotation, per-core inputs."""
    mask = np.asarray(mask)
    m = mask.astype(np.float64)
    ga_hi, ga_lo, gb = _build_tables(W, b)
    tb = _dist_bins(x_t)       # [n, n] int32 in [0, 29]
    sb = _dist_bins(x_sc)

    order = np.argsort(~mask.astype(bool), kind="stable")  # actives first
    n_active = int(mask.astype(bool).sum())
    R = min(128, max(1, (n_active + N_CORES - 1) // N_CORES))

    j = np.arange(1024)
    neg_jhi = (-256.0 * (j // 256))
    neg_jlo = (-(j % 256)).astype(np.float64)

    cores = []
    row_lists = []
    jb_lists = []
    for c in range(N_CORES):
        rows = np.asarray(order[c::N_CORES])  # 128 global row ids
        row_lists.append(rows)
        i_r = rows.astype(np.int64)
        jb = np.clip((i_r - 63) // 128, 0, 6)         # [128] band tile index
        jb_lists.append(jb)
        a = (i_r + 63) // 256
        bb = (i_r + 63) % 256

        # per-row processed->true j permutation (rotation by jb tiles)
        # true_j[r, pos] = ((jb_r + pos//128) % 8)*128 + pos%128
        pos_t = np.arange(1024) // 128
        pos_p = np.arange(1024) % 128
        true_j = (((jb[:, None] + pos_t[None, :]) % 8) * 128 + pos_p[None, :])

        # cols 0..125 map to thresholds k=1..126 -> partitions 0..125 get v
        lta2 = np.zeros((4, 128, 128), np.float64)
        lta2[0, :, 0:126] = a[:, None]
        lta2[1, :, 0:126] = bb[:, None]
        lta2[2, :, 0:126] = 1.0
        lta2[3, :, 0:126] = 1.0
        lta = lta2.reshape(4, 128 * 128)   # [:, r*128+p] = lta2[:, r, p]

        ltb = np.zeros((6, 128, 128), np.float64)
        ltb[0, :, 0:29] = 1.0
        ltb[1, :, 29:58] = 1.0
        ltb[0, :, 58:87] = 1.0
        ltb[1, :, 87:116] = 1.0
        ltb[3, :, 116:118] = 128.0 * jb[:, None]
        ltb[4, :, 116:118] = 1.0
        ltb[5, :, 116:118] = 1.0
        ltb[3, :, 118:120] = -128.0 * jb[:, None]
        ltb[4, :, 118:120] = -1.0
        ltb[5, :, 118:120] = -1.0

        # rowdat: per row 1280 cols = [tbsc block (1024) | A-bcast rhs (256)]
        rowdat = np.zeros((6, 128, 1280), np.float64)
        rowdat[0, :, 0:1024] = tb[i_r[:, None], true_j]
        rowdat[1, :, 0:1024] = sb[i_r[:, None], true_j]
        rowdat[2, :, 0:1024] = 256.0
        rowdat[3, :, 0:1024] = 1.0
        rowdat[4, :, 0:1024] = neg_jhi[true_j]
        rowdat[5, :, 0:1024] = neg_jlo[true_j]
        # A-bcast rhs: window j = [128*jb, 128*jb+256) in natural order
        wj = 128 * jb[:, None] + np.arange(256)[None, :]   # [128, 256]
        rowdat[0, :, 1024:1280] = 256.0
        rowdat[1, :, 1024:1280] = 1.0
        rowdat[2, :, 1024:1280] = neg_jhi[wj]
        rowdat[3, :, 1024:1280] = neg_jlo[wj]

        pmt = np.zeros((128, 1024), np.float32)
        mrow = m[rows]                                  # [128]
        # pmt[p, r*8+t] = mrow[r] * m[true_j[r, t*128+p]]
        mj = m[true_j]                                  # [128 rows, 1024]
        pm_full = mrow[:, None] * mj                    # [128 rows, 1024]
        pmt = np.ascontiguousarray(
            pm_full.reshape(128, 8, 128).transpose(2, 0, 1).reshape(128, 1024)
        ).astype(np.float32)

        cores.append({
            "ga_hi": np.ascontiguousarray(ga_hi),
            "ga_lo": np.ascontiguousarray(ga_lo),
            "gb": np.ascontiguousarray(gb),
            "lta": lta.astype(BF16),
            "ltb": ltb.reshape(6, 128 * 128).astype(BF16),
            "rowdat": rowdat.reshape(6, 128 * 1280).astype(BF16),
            "biasa": _const_biasa(),
            "biasb": _const_biasb(),
            "pmt": pmt,
        })
    return cores, row_lists, jb_lists, R


def _const_biasa():
    v = np.empty((128, 1), np.float32)
    for p in range(126):
        v[p, 0] = -(p + 0.5)     # sign(v - (p+.5)) = +1 iff v >= p+1
    v[126, 0] = 1.0
    v[127, 0] = 1.0
    return v


def _const_biasb():
    v = np.empty((124, 1), np.float32)
    for k in range(29):
        v[k, 0] = -(k + 0.5)
        v[29 + k, 0] = -(k + 0.5)
    v[58:116] = v[0:58]
    v[116:118] = -0.5            # s_plus: +1 iff 128*jb - j >= 1
    v[118:120] = -255.5          # s_minus: +1 iff j - 128*jb >= 256
    v[120:124] = 1.0             # B0 const rows
    return v


def kernel(mask, x_t, x_sc, W, b, gamma, beta):
    global LAST_PROFILE
    from concourse.bass_utils import run_bass_kernel_spmd

    mask = np.asarray(mask)
    cores, row_lists, jb_lists, R = _host_data(mask, x_t, x_sc, W, b)
    nc = _build_program(R, 128 - R)

    trace = bool(int(os.environ.get("KERNEL_TRACE", "0")))
    tdir = os.environ.get("KERNEL_TRACE_DIR") or None
    if tdir:
        os.makedirs(tdir, exist_ok=True)
    res = run_bass_kernel_spmd(nc, cores, list(range(N_CORES)), trace=trace,
                               tmpdir=tdir)
    LAST_PROFILE = res

    out = np.empty((N, N, C_OUT), np.float32)
    for c in range(N_CORES):
        oc = res.results[c]["out"]          # [128, 1024, 256] rotated rows
        rows = row_lists[c]
        jb = jb_lists[c]
        for r in range(128):
            if r < R and jb[r]:
                out[rows[r]] = np.roll(
                    oc[r].reshape(8, 128, C_OUT), jb[r], axis=0
                ).reshape(1024, C_OUT)
            else:
                out[rows[r]] = oc[r]

    gamma = np.asarray(gamma, np.float32)
    beta = np.asarray(beta, np.float32)
    if not (np.all(gamma == 1.0) and np.all(beta == 0.0)):
        pm = (mask.astype(np.float32)[:, None] * mask.astype(np.float32)[None, :])
        out = out * gamma[None, None, :] + pm[:, :, None] * beta[None, None, :]
    return out



# revision 5
# speedup vs baseline: 1.6561x; 1.6561x over previous
"""Trainium2 Bass kernel for nn_DenoiserPairFeatures (v2).

Math: the [n,n,219] feature tensor is a concat of one-hots (seq-sep 127,
dist-bins 30+30) plus zero blocks, so feats @ W.T + b collapses to 3 table
gathers + bias, realized on the TensorEngine as 0/1-indicator matmuls
against host-precomputed compensated-cumulative bf16 tables.

v2 structure:
- Mask sparsity: only active rows (mask[i]=1) are computed, round-robin
  over 8 cores; only active columns are computed, packed densely into
  JTa j-tiles shared by all rows.  Host scatters results into the full
  [n,n,256] output (inactive pairs are exactly zero).
- LayerNorm mean is free: all table rows are mean-centered on host, so
  the gathered y is already mean-subtracted (linearity).  Only E[y^2]
  is reduced on device.
- The indicator matrices F are built by a single DVE compare
  (tensor_scalar is_gt) against per-partition thresholds, fed by small
  broadcast DMAs - no PE/Act work.
- Per 128-pair tile: PE matmul (K=126 band chain + K=62 bins chain) ->
  PSUM f32; Act copies PSUM->SBUF bf16; DVE square-reduces for E[y^2];
  Act sqrt + DVE reciprocal; DVE applies out = y * rsqrt(var+eps); DMA
  out in bf16 (host converts to f32).
"""

import os
import sys

sys.path.insert(0, "/opt/trn_rl_repo")

import numpy as np
import ml_dtypes

N = 1024
SEQ = 127          # seq-sep one-hot classes
NB = 30            # dist bins
C_OUT = 256
N_CORES = 8
LN_EPS = 1e-5
KB = 62            # B-matrix rows: 29 t + 29 sc + 2 sep-left + 2 bias
KA = 126           # A-matrix rows: sep chain

BF16 = ml_dtypes.bfloat16

_PROGRAM_CACHE = {}
LAST_PROFILE = None  # set when KERNEL_TRACE=1


def _bf16_f64(x):
    return np.asarray(x, np.float64).astype(BF16).astype(np.float64)


def _comp_chain01(T):
    """0/1-indicator compensated chain, single-level bf16.

    T: [M+1, C] float64 targets.  inc[k] bf16-realized so that partial sums
    sum(inc[0:s]) track T[s]-T[0] with non-accumulating ~bf16 error."""
    M = T.shape[0] - 1
    C = T.shape[1]
    P = np.zeros(C, np.float64)
    inc = np.empty((M, C), np.float64)
    for k in range(M):
        g = T[k + 1] - T[0] - P
        gq = _bf16_f64(g)
        inc[k] = gq
        P += gq
    return inc


def _dist_bins(coords):
    """Bin indices exactly as the reference computes them."""
    import jax.numpy as jnp

    edges = jnp.linspace(0.1, 3.0, NB - 1)
    x = jnp.asarray(np.asarray(coords, np.float32))
    diff = x[:, None, :] - x[None, :, :]
    d = jnp.sqrt(jnp.sum(jnp.square(diff), axis=-1) + 1e-10)
    return np.asarray(jnp.searchsorted(edges, d), dtype=np.int32)


def _build_tables(W, b):
    """Mean-centered chain tables.  ga [126,256] sep chain; gb [62,256]:
    t-chain 29, sc-chain 29, sep (T126-T0) hi+lo, base hi+lo."""
    W = np.asarray(W, np.float64)
    b = np.asarray(b, np.float64)
    Tsep = W[:, 0:SEQ].T.copy()
    Tt = W[:, SEQ:SEQ + NB].T.copy()
    Tsc = W[:, SEQ + NB:SEQ + 2 * NB].T.copy()
    Tsep -= Tsep.mean(axis=1, keepdims=True)
    Tt -= Tt.mean(axis=1, keepdims=True)
    Tsc -= Tsc.mean(axis=1, keepdims=True)
    bc = b - b.mean()

    ga = _comp_chain01(Tsep)                    # [126, 256]
    incT = _comp_chain01(Tt)                    # [29, 256]
    incS = _comp_chain01(Tsc)                   # [29, 256]
    diff = Tsep[SEQ - 1] - Tsep[0]
    d_hi = _bf16_f64(diff)
    d_lo = _bf16_f64(diff - d_hi)
    B0 = bc + Tsep[0] + Tt[0] + Tsc[0]
    b_hi = _bf16_f64(B0)
    b_lo = _bf16_f64(B0 - b_hi)
    gb = np.concatenate(
        [incT, incS, d_hi[None], d_lo[None], b_hi[None], b_lo[None]], axis=0)
    return ga.astype(BF16), gb.astype(BF16)


def _thresholds():
    tha = np.empty((KA, 1), np.float32)
    for p in range(KA):
        tha[p, 0] = p - 62.5            # F_A[p] = (i-j > p-62.5)
    thb = np.empty((KB, 1), np.float32)
    for k in range(29):
        thb[k, 0] = k + 0.5             # (tb > k+0.5)
        thb[29 + k, 0] = k + 0.5        # (sb > k+0.5)
    thb[58:60, 0] = 0.5                 # (-v > 0.5)  <=> p < 128*pb
    thb[60:62, 0] = 0.5                 # (1 > 0.5)   constant rows
    return tha, thb


def _build_program(R, JT):
    """Build + compile the SPMD program for R row-slots, JT packed j-tiles."""
    key = (R, JT)
    if key in _PROGRAM_CACHE:
        return _PROGRAM_CACHE[key]

    from concourse import bacc, mybir, tile

    P = JT * 128
    dt = mybir.dt
    nc = bacc.Bacc("TRN2", target_bir_lowering=False, debug=False,
                   num_devices=N_CORES)

    ga_d = nc.dram_tensor("ga", [KA, C_OUT], dt.bfloat16, kind="ExternalInput").ap()
    gb_d = nc.dram_tensor("gb", [KB, C_OUT], dt.bfloat16, kind="ExternalInput").ap()
    tha_d = nc.dram_tensor("tha", [KA, 1], dt.float32, kind="ExternalInput").ap()
    thb_d = nc.dram_tensor("thb", [KB, 1], dt.float32, kind="ExternalInput").ap()
    bcb_d = nc.dram_tensor("bcb", [4, R * P], dt.bfloat16, kind="ExternalInput").ap()
    bca_d = nc.dram_tensor("bca", [1, R * 256], dt.bfloat16, kind="ExternalInput").ap()
    out_d = nc.dram_tensor("out", [R, P, C_OUT], dt.bfloat16,
                           kind="ExternalOutput").ap()

    NBANK = (JT + 1) // 2

    with tile.TileContext(nc) as tc:
        with (
            tc.tile_pool(name="const", bufs=1) as cpool,
            tc.tile_pool(name="bc", bufs=3) as bcpool,
            tc.tile_pool(name="f", bufs=3) as fpool,
            tc.tile_pool(name="y", bufs=4, space="PSUM") as ypool,
            tc.tile_pool(name="yh", bufs=3) as yhpool,
            tc.tile_pool(name="st", bufs=4) as stpool,
            tc.tile_pool(name="ot", bufs=3) as opool,
        ):
            GA = cpool.tile([KA, C_OUT], dt.bfloat16)
            nc.sync.dma_start(out=GA[:], in_=ga_d[:])
            GB = cpool.tile([KB, C_OUT], dt.bfloat16)
            nc.sync.dma_start(out=GB[:], in_=gb_d[:])
            THA = cpool.tile([KA, 1], dt.float32)
            nc.sync.dma_start(out=THA[:], in_=tha_d[:])
            THB = cpool.tile([KB, 1], dt.float32)
            nc.sync.dma_start(out=THB[:], in_=thb_d[:])
            EPS = cpool.tile([128, 1], dt.float32)
            nc.vector.memset(EPS[:], LN_EPS)
            SQ = cpool.tile([128, C_OUT], dt.bfloat16)  # ttr waste-write scratch

            Sqrt = mybir.ActivationFunctionType.Sqrt
            Copy = mybir.ActivationFunctionType.Copy
            mult = mybir.AluOpType.mult
            add = mybir.AluOpType.add
            is_gt = mybir.AluOpType.is_gt

            for r in range(R):
                # ---- stage per-row compare inputs (partition-broadcast) ----
                BCB = bcpool.tile([KB, P], dt.bfloat16, tag="bcb")
                src = bcb_d[:, r * P:(r + 1) * P]
                nc.sync.dma_start(out=BCB[0:29, :],
                                  in_=src[0:1, :].to_broadcast([29, P]))
                nc.sync.dma_start(out=BCB[29:58, :],
                                  in_=src[1:2, :].to_broadcast([29, P]))
                nc.sync.dma_start(out=BCB[58:60, :],
                                  in_=src[2:3, :].to_broadcast([2, P]))
                nc.sync.dma_start(out=BCB[60:62, :],
                                  in_=src[3:4, :].to_broadcast([2, P]))
                BCA = bcpool.tile([KA, 256], dt.bfloat16, tag="bca")
                nc.sync.dma_start(
                    out=BCA[:],
                    in_=bca_d[0:1, r * 256:(r + 1) * 256].to_broadcast([KA, 256]))

                # ---- indicator matrices via DVE compare ----
                FB = fpool.tile([KB, P], dt.bfloat16, tag="fb")
                nc.vector.tensor_scalar(FB[:], BCB[:], THB[:, 0:1], None, op0=is_gt)
                FA = fpool.tile([KA, 256], dt.bfloat16, tag="fa")
                nc.vector.tensor_scalar(FA[:], BCA[:], THA[:, 0:1], None, op0=is_gt)

                # ---- per-bank: matmuls -> copy -> square-reduce ----
                YH = yhpool.tile([128, JT * C_OUT], dt.bfloat16, tag="yh")
                Q = stpool.tile([128, JT], dt.float32, tag="q")
                for bank in range(NBANK):
                    ns = min(2, JT - 2 * bank)
                    Y = ypool.tile([128, 2, C_OUT], dt.float32, tag="y")
                    for s in range(ns):
                        q = 2 * bank + s
                        if q < 2:
                            nc.tensor.matmul(Y[:, s, :],
                                             FA[:, q * 128:(q + 1) * 128],
                                             GA[:], start=True, stop=False)
                            nc.tensor.matmul(Y[:, s, :],
                                             FB[:, q * 128:(q + 1) * 128],
                                             GB[:], start=False, stop=True)
                        else:
                            nc.tensor.matmul(Y[:, s, :],
                                             FB[:, q * 128:(q + 1) * 128],
                                             GB[:], start=True, stop=True)
                    nc.scalar.activation(
                        YH[:, 2 * bank * C_OUT:(2 * bank + ns) * C_OUT],
                        Y[:, 0:ns, :], Copy)
                    for s in range(ns):
                        q = 2 * bank + s
                        nc.vector.scalar_tensor_tensor(
                            SQ[:],
                            YH[:, q * C_OUT:(q + 1) * C_OUT], 1.0,
                            YH[:, q * C_OUT:(q + 1) * C_OUT],
                            op0=mult, op1=mult,
                            accum_out=Q[:, q:q + 1])

                # ---- rstd + apply ----
                SD1 = stpool.tile([128, JT], dt.float32, tag="sd1")
                nc.scalar.activation(SD1[:], Q[:], Sqrt, bias=EPS[:, 0:1],
                                     scale=1.0 / C_OUT)
                SD = stpool.tile([128, JT], dt.float32, tag="sd")
                nc.vector.reciprocal(SD[:], SD1[:])
                OT = opool.tile([128, JT * C_OUT], dt.bfloat16, tag="ot")
                for q in range(JT):
                    nc.vector.tensor_scalar(
                        OT[:, q * C_OUT:(q + 1) * C_OUT],
                        YH[:, q * C_OUT:(q + 1) * C_OUT],
                        SD[:, q:q + 1], None, op0=mult)

                # ---- output DMA (split for queue parallelism) ----
                for h in range(NBANK):
                    ns = min(2, JT - 2 * h)
                    nc.sync.dma_start(
                        out=out_d[r, h * 256:h * 256 + ns * 128, :]
                            .rearrange("(q p) o -> p q o", p=128),
                        in_=OT[:, 2 * h * C_OUT:(2 * h + ns) * C_OUT]
                            .rearrange("p (q o) -> p q o", o=C_OUT))

    nc.compile()
    _PROGRAM_CACHE[key] = nc
    return nc


def _host_data(mask, x_t, x_sc, W, b):
    """Active-row/col packing, per-core compare inputs, tables."""
    mask = np.asarray(mask)
    act = mask != 0
    A = np.flatnonzero(act)
    nA = int(len(A))
    if nA == 0:
        return None
    JT = max(2, (nA + 127) // 128)
    P = JT * 128
    Ap = np.concatenate([A, np.full(P - nA, A[-1], dtype=A.dtype)])
    Rc = (nA + N_CORES - 1) // N_CORES

    ga, gb = _build_tables(W, b)
    tha, thb = _thresholds()
    tb = _dist_bins(x_t)
    sb = _dist_bins(x_sc)

    pos = np.arange(P)
    pos_t = pos // 128
    pos_p = pos % 128

    cores = []
    meta = []
    for c in range(N_CORES):
        rows_c = A[c::N_CORES]
        nr = len(rows_c)
        rows = np.full(Rc, rows_c[-1] if nr else A[0], dtype=np.int64)
        rows[:nr] = rows_c

        lo = np.searchsorted(A, rows - 62, side="left")
        pb = np.clip(lo // 128, 0, JT - 2)

        # processed position -> packed index (rotation by pb tiles)
        ptrue = ((pb[:, None] + pos_t[None, :]) % JT) * 128 + pos_p[None, :]
        jtrue = Ap[ptrue]                          # [Rc, P] true col ids

        tbv = tb[rows[:, None], jtrue]             # [Rc, P]
        sbv = sb[rows[:, None], jtrue]
        vv = ptrue - 128 * pb[:, None]             # packed idx - window start
        bcb = np.empty((4, Rc, P), np.float64)
        bcb[0] = tbv
        bcb[1] = sbv
        bcb[2] = -vv
        bcb[3] = 1.0

        jwin = jtrue[:, 0:256]                     # window cols (natural order)
        u = rows[:, None] - jwin                   # i - j
        bca = u.astype(np.float64)

        cores.append({
            "ga": np.ascontiguousarray(ga),
            "gb": np.ascontiguousarray(gb),
            "tha": tha,
            "thb": thb,
            "bcb": bcb.reshape(4, Rc * P).astype(BF16),
            "bca": bca.reshape(1, Rc * 256).astype(BF16),
        })
        meta.append((rows_c, pb[:nr] if nr else pb[:0]))
    return cores, meta, A, nA, Rc, JT


def kernel(mask, x_t, x_sc, W, b, gamma, beta):
    global LAST_PROFILE
    from concourse.bass_utils import run_bass_kernel_spmd

    mask = np.asarray(mask)
    out = np.zeros((N, N, C_OUT), np.float32)
    host = _host_data(mask, x_t, x_sc, W, b)
    if host is not None:
        cores, meta, A, nA, Rc, JT = host
        P = JT * 128
        nc = _build_program(Rc, JT)

        trace = bool(int(os.environ.get("KERNEL_TRACE", "0")))
        tdir = os.environ.get("KERNEL_TRACE_DIR") or None
        if tdir:
            os.makedirs(tdir, exist_ok=True)
        res = run_bass_kernel_spmd(nc, cores, list(range(N_CORES)), trace=trace,
                                   tmpdir=tdir)
        LAST_PROFILE = res

        for c in range(N_CORES):
            oc = res.results[c]["out"]             # [Rc, P, 256] bf16
            rows_c, pbs = meta[c]
            for r, (i, pbr) in enumerate(zip(rows_c, pbs)):
                blk = oc[r].reshape(JT, 128, C_OUT)
                if pbr:
                    blk = np.roll(blk, pbr, axis=0)
                out[i, A] = blk.reshape(P, C_OUT)[:nA].astype(np.float32)

    gamma = np.asarray(gamma, np.float32)
    beta = np.asarray(beta, np.float32)
    if not (np.all(gamma == 1.0) and np.all(beta == 0.0)):
        pm = (mask.astype(np.float32)[:, None] * mask.astype(np.float32)[None, :])
        out = out * gamma[None, None, :] + pm[:, :, None] * beta[None, None, :]
    return out


# revision 11
# speedup vs baseline: 1.9872x; 1.2000x over previous
"""Trainium2 Bass kernel for nn_DenoiserPairFeatures (v2).

Math: the [n,n,219] feature tensor is a concat of one-hots (seq-sep 127,
dist-bins 30+30) plus zero blocks, so feats @ W.T + b collapses to 3 table
gathers + bias, realized on the TensorEngine as 0/1-indicator matmuls
against host-precomputed compensated-cumulative bf16 tables.

v2 structure:
- Mask sparsity: only active rows (mask[i]=1) are computed, round-robin
  over 8 cores; only active columns are computed, packed densely into
  JTa j-tiles shared by all rows.  Host scatters results into the full
  [n,n,256] output (inactive pairs are exactly zero).
- LayerNorm mean is free: all table rows are mean-centered on host, so
  the gathered y is already mean-subtracted (linearity).  Only E[y^2]
  is reduced on device.
- The indicator matrices F are built by a single DVE compare
  (tensor_scalar is_gt) against per-partition thresholds, fed by small
  broadcast DMAs - no PE/Act work.
- Per 128-pair tile: PE matmul (K=126 band chain + K=62 bins chain) ->
  PSUM f32; Act copies PSUM->SBUF bf16; DVE square-reduces for E[y^2];
  Act sqrt + DVE reciprocal; DVE applies out = y * rsqrt(var+eps); DMA
  out in bf16 (host converts to f32).
"""

import os
import sys

sys.path.insert(0, "/opt/trn_rl_repo")

import numpy as np
import ml_dtypes

N = 1024
SEQ = 127          # seq-sep one-hot classes
NB = 30            # dist bins
C_OUT = 256
N_CORES = 8
LN_EPS = 1e-5
KB = 62            # B-matrix rows: 29 t + 29 sc + 2 sep-left + 2 bias
KA = 126           # A-matrix rows: sep chain

BF16 = ml_dtypes.bfloat16

_PROGRAM_CACHE = {}
LAST_PROFILE = None  # set when KERNEL_TRACE=1


def _bf16_f64(x):
    return np.asarray(x, np.float64).astype(BF16).astype(np.float64)


def _comp_chain01(T):
    """0/1-indicator compensated chain, single-level bf16.

    T: [M+1, C] float64 targets.  inc[k] bf16-realized so that partial sums
    sum(inc[0:s]) track T[s]-T[0] with non-accumulating ~bf16 error."""
    M = T.shape[0] - 1
    C = T.shape[1]
    P = np.zeros(C, np.float64)
    inc = np.empty((M, C), np.float64)
    for k in range(M):
        g = T[k + 1] - T[0] - P
        gq = _bf16_f64(g)
        inc[k] = gq
        P += gq
    return inc


def _dist_bins(coords):
    """Bin indices exactly as the reference computes them."""
    import jax.numpy as jnp

    edges = jnp.linspace(0.1, 3.0, NB - 1)
    x = jnp.asarray(np.asarray(coords, np.float32))
    diff = x[:, None, :] - x[None, :, :]
    d = jnp.sqrt(jnp.sum(jnp.square(diff), axis=-1) + 1e-10)
    return np.asarray(jnp.searchsorted(edges, d), dtype=np.int32)


def _build_tables(W, b):
    """Mean-centered chain tables.  ga [126,256] sep chain; gb [62,256]:
    t-chain 29, sc-chain 29, sep (T126-T0) hi+lo, base hi+lo."""
    W = np.asarray(W, np.float64)
    b = np.asarray(b, np.float64)
    Tsep = W[:, 0:SEQ].T.copy()
    Tt = W[:, SEQ:SEQ + NB].T.copy()
    Tsc = W[:, SEQ + NB:SEQ + 2 * NB].T.copy()
    Tsep -= Tsep.mean(axis=1, keepdims=True)
    Tt -= Tt.mean(axis=1, keepdims=True)
    Tsc -= Tsc.mean(axis=1, keepdims=True)
    bc = b - b.mean()

    ga = _comp_chain01(Tsep)                    # [126, 256]
    incT = _comp_chain01(Tt)                    # [29, 256]
    incS = _comp_chain01(Tsc)                   # [29, 256]
    diff = Tsep[SEQ - 1] - Tsep[0]
    d_hi = _bf16_f64(diff)
    d_lo = _bf16_f64(diff - d_hi)
    B0 = bc + Tsep[0] + Tt[0] + Tsc[0]
    b_hi = _bf16_f64(B0)
    b_lo = _bf16_f64(B0 - b_hi)
    gb = np.concatenate(
        [incT, incS, d_hi[None], d_lo[None], b_hi[None], b_lo[None]], axis=0)
    return ga.astype(BF16), gb.astype(BF16)


def _thresholds():
    tha = np.empty((KA, 1), np.float32)
    for p in range(KA):
        tha[p, 0] = p - 62.5            # F_A[p] = (i-j > p-62.5)
    thb = np.empty((KB, 1), np.float32)
    for k in range(29):
        thb[k, 0] = k + 0.5             # (tb > k+0.5)
        thb[29 + k, 0] = k + 0.5        # (sb > k+0.5)
    thb[58:60, 0] = 0.5                 # (-v > 0.5)  <=> p < 128*pb
    thb[60:62, 0] = 0.5                 # (1 > 0.5)   constant rows
    return tha, thb


def _build_program(R, JT):
    """Build + compile the SPMD program for R row-slots, JT packed j-tiles."""
    key = (R, JT)
    if key in _PROGRAM_CACHE:
        return _PROGRAM_CACHE[key]

    from concourse import bacc, mybir, tile

    P = JT * 128
    dt = mybir.dt
    nc = bacc.Bacc("TRN2", target_bir_lowering=False, debug=False,
                   num_devices=N_CORES)

    ga_d = nc.dram_tensor("ga", [KA, C_OUT], dt.bfloat16, kind="ExternalInput").ap()
    gb_d = nc.dram_tensor("gb", [KB, C_OUT], dt.bfloat16, kind="ExternalInput").ap()
    tha_d = nc.dram_tensor("tha", [KA, 1], dt.float32, kind="ExternalInput").ap()
    thb_d = nc.dram_tensor("thb", [KB, 1], dt.float32, kind="ExternalInput").ap()
    bcb_d = nc.dram_tensor("bcb", [R * KB, P], dt.bfloat16, kind="ExternalInput").ap()
    bca_d = nc.dram_tensor("bca", [R * KA, 256], dt.bfloat16, kind="ExternalInput").ap()
    # partition-major output: addr = ((r*128+p)*JT + q)*C_OUT + o
    out_d = nc.dram_tensor("out", [R, 128, JT * C_OUT], dt.bfloat16,
                           kind="ExternalOutput").ap()

    NBANK = (JT + 1) // 2

    with tile.TileContext(nc) as tc:
        with (
            tc.tile_pool(name="const", bufs=1) as cpool,
            tc.tile_pool(name="bc", bufs=3) as bcpool,
            tc.tile_pool(name="f", bufs=3) as fpool,
            tc.tile_pool(name="y", bufs=4, space="PSUM") as ypool,
            tc.tile_pool(name="st", bufs=4) as stpool,
            tc.tile_pool(name="ot", bufs=3) as opool,
        ):
            GA = cpool.tile([KA, C_OUT], dt.bfloat16)
            nc.sync.dma_start(out=GA[:], in_=ga_d[:])
            GB = cpool.tile([KB, C_OUT], dt.bfloat16)
            nc.sync.dma_start(out=GB[:], in_=gb_d[:])
            THA = cpool.tile([KA, 1], dt.float32)
            nc.sync.dma_start(out=THA[:], in_=tha_d[:])
            THB = cpool.tile([KB, 1], dt.float32)
            nc.sync.dma_start(out=THB[:], in_=thb_d[:])
            EPS = cpool.tile([128, 1], dt.float32)
            nc.vector.memset(EPS[:], LN_EPS)
            SQ = cpool.tile([128, C_OUT], dt.bfloat16)  # ttr waste-write scratch

            Sqrt = mybir.ActivationFunctionType.Sqrt
            Square = mybir.ActivationFunctionType.Square
            mult = mybir.AluOpType.mult
            is_gt = mybir.AluOpType.is_gt

            for r in range(R):
                # ---- stage per-row compare inputs (pre-expanded) ----
                BCB = bcpool.tile([KB, P], dt.bfloat16, tag="bcb")
                nc.sync.dma_start(out=BCB[:], in_=bcb_d[r * KB:(r + 1) * KB, :])
                BCA = bcpool.tile([KA, 256], dt.bfloat16, tag="bca")
                nc.sync.dma_start(out=BCA[:], in_=bca_d[r * KA:(r + 1) * KA, :])

                # ---- indicator matrices via DVE compare ----
                FB = fpool.tile([KB, P], dt.bfloat16, tag="fb")
                nc.vector.tensor_scalar(FB[:], BCB[:], THB[:, 0:1], None, op0=is_gt)
                FA = fpool.tile([KA, 256], dt.bfloat16, tag="fa")
                nc.vector.tensor_scalar(FA[:], BCA[:], THA[:, 0:1], None, op0=is_gt)

                # ---- per-bank: matmuls; Act square+accum from PSUM ----
                ytiles = []
                Q = stpool.tile([128, JT], dt.float32, tag="q")
                for bank in range(NBANK):
                    ns = min(2, JT - 2 * bank)
                    Y = ypool.tile([128, 2, C_OUT], dt.float32, tag="y")
                    ytiles.append(Y)
                    for s in range(ns):
                        q = 2 * bank + s
                        if q < 2:
                            nc.tensor.matmul(Y[:, s, :],
                                             FA[:, q * 128:(q + 1) * 128],
                                             GA[:], start=True, stop=False)
                            nc.tensor.matmul(Y[:, s, :],
                                             FB[:, q * 128:(q + 1) * 128],
                                             GB[:], start=False, stop=True)
                        else:
                            nc.tensor.matmul(Y[:, s, :],
                                             FB[:, q * 128:(q + 1) * 128],
                                             GB[:], start=True, stop=True)
                        nc.scalar.activation(SQ[:], Y[:, s, :], Square,
                                             accum_out=Q[:, q:q + 1])

                # ---- rstd + apply ----
                SD1 = stpool.tile([128, JT], dt.float32, tag="sd1")
                nc.scalar.activation(SD1[:], Q[:], Sqrt, bias=EPS[:, 0:1],
                                     scale=1.0 / C_OUT)
                SD = stpool.tile([128, JT], dt.float32, tag="sd")
                nc.vector.reciprocal(SD[:], SD1[:])
                OT = opool.tile([128, JT * C_OUT], dt.bfloat16, tag="ot")
                for q in range(JT):
                    nc.vector.tensor_scalar(
                        OT[:, q * C_OUT:(q + 1) * C_OUT],
                        ytiles[q // 2][:, q % 2, :],
                        SD[:, q:q + 1], None, op0=mult)

                # ---- output DMA: partition-major, contiguous per partition ----
                nc.sync.dma_start(out=out_d[r], in_=OT[:])

    nc.compile()
    _PROGRAM_CACHE[key] = nc
    return nc


def _host_data(mask, x_t, x_sc, W, b):
    """Active-row/col packing, per-core compare inputs, tables."""
    mask = np.asarray(mask)
    act = mask != 0
    A = np.flatnonzero(act)
    nA = int(len(A))
    if nA == 0:
        return None
    JT = max(2, (nA + 127) // 128)
    P = JT * 128
    Ap = np.concatenate([A, np.full(P - nA, A[-1], dtype=A.dtype)])
    Rc = (nA + N_CORES - 1) // N_CORES

    ga, gb = _build_tables(W, b)
    tha, thb = _thresholds()
    tb = _dist_bins(x_t)
    sb = _dist_bins(x_sc)

    pos = np.arange(P)
    pos_t = pos // 128
    pos_p = pos % 128

    cores = []
    meta = []
    for c in range(N_CORES):
        rows_c = A[c::N_CORES]
        nr = len(rows_c)
        rows = np.full(Rc, rows_c[-1] if nr else A[0], dtype=np.int64)
        rows[:nr] = rows_c

        lo = np.searchsorted(A, rows - 62, side="left")
        pb = np.clip(lo // 128, 0, JT - 2)

        # processed position -> packed index (rotation by pb tiles)
        ptrue = ((pb[:, None] + pos_t[None, :]) % JT) * 128 + pos_p[None, :]
        jtrue = Ap[ptrue]                          # [Rc, P] true col ids

        tbv = tb[rows[:, None], jtrue]             # [Rc, P]
        sbv = sb[rows[:, None], jtrue]
        vv = ptrue - 128 * pb[:, None]             # packed idx - window start
        bcb = np.empty((Rc, KB, P), np.float32)
        bcb[:, 0:29, :] = tbv[:, None, :]
        bcb[:, 29:58, :] = sbv[:, None, :]
        bcb[:, 58:60, :] = -vv[:, None, :]
        bcb[:, 60:62, :] = 1.0

        jwin = jtrue[:, 0:256]                     # window cols (natural order)
        u = (rows[:, None] - jwin).astype(np.float32)  # i - j
        bca = np.broadcast_to(u[:, None, :], (Rc, KA, 256))

        cores.append({
            "ga": np.ascontiguousarray(ga),
            "gb": np.ascontiguousarray(gb),
            "tha": tha,
            "thb": thb,
            "bcb": np.ascontiguousarray(bcb.reshape(Rc * KB, P)).astype(BF16),
            "bca": np.ascontiguousarray(bca.reshape(Rc * KA, 256)).astype(BF16),
        })
        meta.append((rows_c, pb[:nr] if nr else pb[:0]))
    return cores, meta, A, nA, Rc, JT


def kernel(mask, x_t, x_sc, W, b, gamma, beta):
    global LAST_PROFILE
    from concourse.bass_utils import run_bass_kernel_spmd

    mask = np.asarray(mask)
    out = np.zeros((N, N, C_OUT), np.float32)
    host = _host_data(mask, x_t, x_sc, W, b)
    if host is not None:
        cores, meta, A, nA, Rc, JT = host
        P = JT * 128
        nc = _build_program(Rc, JT)

        trace = bool(int(os.environ.get("KERNEL_TRACE", "0")))
        tdir = os.environ.get("KERNEL_TRACE_DIR") or None
        if tdir:
            os.makedirs(tdir, exist_ok=True)
        res = run_bass_kernel_spmd(nc, cores, list(range(N_CORES)), trace=trace,
                                   tmpdir=tdir)
        LAST_PROFILE = res

        for c in range(N_CORES):
            oc = res.results[c]["out"]             # [Rc, 128, JT*256] bf16
            rows_c, pbs = meta[c]
            for r, (i, pbr) in enumerate(zip(rows_c, pbs)):
                blk = oc[r].reshape(128, JT, C_OUT).transpose(1, 0, 2)
                if pbr:
                    blk = np.roll(blk, pbr, axis=0)
                out[i, A] = blk.reshape(P, C_OUT)[:nA].astype(np.float32)

    gamma = np.asarray(gamma, np.float32)
    beta = np.asarray(beta, np.float32)
    if not (np.all(gamma == 1.0) and np.all(beta == 0.0)):
        pm = (mask.astype(np.float32)[:, None] * mask.astype(np.float32)[None, :])
        out = out * gamma[None, None, :] + pm[:, :, None] * beta[None, None, :]
    return out
